# revision 13
# baseline (speedup 1.0000x reference)
"""Trainium2 Bass kernel for a 2-layer DGCN (graph conv) on 8 NeuronCores.

Reference computation (fp32):
    h1  = relu(IFadj @ (x @ W1) + b1)         # [N, NHID]
    out = BN(adj @ (h1 @ W2) + b2)            # [N, OUTD], BN in eval mode

Distribution: rows of IFadj / adj (= output rows) are sharded across 8
cores; weights are replicated. S = x @ W1 is split hybrid-style:

  - each core computes its own first GP=2 i-blocks of S and AllGathers
    them (doorbell fires ~6us into the kernel, so the collective executes
    the moment the NRT CC-stream bootstrap barrier ends, ~70-95us in --
    measured, the first collective cannot execute earlier than that no
    matter when it is issued);
  - the remaining NREP=6 i-blocks of EVERY core-group (48 blocks) are
    computed redundantly on every core (fp8 DoubleRow, ~52us). This
    fills the whole bootstrap window with work instead of stalling on
    the gather (a pure-gather design idles ~40-70us; a pure-replication
    design pays ~37us more dense compute and, having no early
    collective, pays the multi-core start skew at the mid-kernel Z
    gather instead).

  The early S AllGather also wall-clock-synchronizes the cores, so the
  two mid-kernel Z AllGathers run at pure transfer cost (~12us, hidden).

Per core k (rows R_k):
    phase 1: S-own (2 blocks) -> bounce -> AllGather; S-rep (48 blocks)
    phase 2: h1T = relu(S^T @ BshT_k + b1eff)   (h1 transposed, [NHID, ROWS])
             in two i-half passes, replicated-S pairs first, gathered
             pairs last; after each half, z rows for that half are
             produced and the Z-AllGather chunk is issued mid-kernel.
    phase 3: z_k = h1T.T @ (W2/2)         -> AllGather Z (2 pipelined chunks)
    phase 4: outT = Z-as-lhsT vs adjT_k rhs -> [OUTD, ROWS], fused BN in
             the PSUM-evict op; the host transposes per-core outputs.

fp8 scheme (rel-err budget 2e-2; measured ~2.3e-3, same as all-bf16):
  All three big matmuls run in fp8e4m3 with perf_mode=DoubleRow (two
  128-row contraction tiles per matmul, ~1.9x tensor throughput, half
  the DMA/collective bytes). Naive e4m3 on uniform[0,1) adjacency loses
  ~1.7e-2 rel err via a per-column bias in h1 that the next adjacency
  matmul amplifies ~4096x; instead IFadj is mean-shifted on the host
  (B = IFadj - 0.5, in [-.5,.5)) and the exact rank-1 correction
  0.5*colsum(x @ W1) = 0.5*(colsum(x) @ W1) is folded into the relu
  bias -- computed exactly on the host (this exactness matters: the
  correction also cancels the column-bias of quantizing S). W1 is
  pre-scaled 8x into the e4m3 normal range (PSUM evict undoes it), and
  W2 by 0.5 so |Z|<120 stays far from the TRN e4m3 max of 240 (the BN
  scale is doubled to undo that). h1 stays bf16 and W2 bf16: quantizing
  W2 puts a column-bias on Z that L2's adjacency matmul amplifies.

The PE consumes the left operand transposed (out = lhsT.T @ rhs), so the
host passes IFadj[R_k].T / adj[R_k].T / x-slices.T per core; with the
h1T / outT formulations no on-device transposes are needed anywhere.
Accumulation everywhere is fp32.
"""

import numpy as np
import ml_dtypes

NCORES = 8
N = 8192
NFEAT = 1024
NHID = 512
OUTD = 256
ROWS = N // NCORES  # 1024
P = 128
BN_EPS = 1e-5

CB = NFEAT // P   # 8  c-blocks (x feature contraction)
IB = ROWS // P    # 8  i-blocks (local rows)
JB = NHID // P    # 4  j-blocks (hidden)
MT = N // P       # 64 m-tiles (global node contraction)
HF = 512          # matmul moving free dim (PSUM bank limit)
IH = ROWS // HF   # 2 i-halves of the local row range
OB = OUTD // P    # 2 output-feature blocks
GC = 2            # Z allgather chunks (one per i-half)
QT = 4            # m-tiles per (core-block, chunk) = IB // GC
GP = 4            # i-blocks per core gathered (the rest replicated)
NREP = IB - GP    # i-blocks per core-group replicated on every core (4)

_BF16 = ml_dtypes.bfloat16
_FP8 = ml_dtypes.float8_e4m3  # TRN fp8e4 (IEEE-style, max 240)

_cache = {}


def _build():
    import concourse.mybir as mybir
    import concourse.tile as tile
    from concourse import bacc

    dt = mybir.dt
    f32 = dt.float32
    bf16 = dt.bfloat16
    fp8 = dt.float8e4
    AF = mybir.ActivationFunctionType
    DR = mybir.MatmulPerfMode.DoubleRow

    nc = bacc.Bacc("TRN2", target_bir_lowering=False, debug=False,
                   num_devices=NCORES)

    # xTo: core's own first GP i-blocks (gathered); xTr: the NREP-per-group
    # replicated blocks, identical on every core, packed r = NREP*k + (j-GP)
    xTo_e = nc.dram_tensor("xTo", [NFEAT, GP * P], fp8, kind="ExternalInput")
    xTr_e = nc.dram_tensor("xTr", [NFEAT, NREP * NCORES * P], fp8,
                           kind="ExternalInput")
    ifadjT_e = nc.dram_tensor("ifadjT", [N, ROWS], fp8, kind="ExternalInput")
    adjT_e = nc.dram_tensor("adjT", [N, ROWS], fp8, kind="ExternalInput")
    w1_e = nc.dram_tensor("w1", [NFEAT, NHID], fp8, kind="ExternalInput")
    w2_e = nc.dram_tensor("w2", [NHID, OUTD], bf16, kind="ExternalInput")
    b1p_e = nc.dram_tensor("b1p", [P, JB], f32, kind="ExternalInput")
    bnsc_e = nc.dram_tensor("bnsc", [P, OB], f32, kind="ExternalInput")
    bnbi_e = nc.dram_tensor("bnbi", [P, OB], f32, kind="ExternalInput")
    # outT: [OUTD, ROWS]; the host transposes each core's block.
    out_e = nc.dram_tensor("out", [OUTD, ROWS], f32, kind="ExternalOutput")

    groups = [list(range(NCORES))]

    def allgather(g_in, g_out):
        nc.gpsimd.collective_compute(
            "AllGather", mybir.AluOpType.bypass, replica_groups=groups,
            ins=[g_in[:]], outs=[g_out[:]])

    with tile.TileContext(nc) as tc:
        with (
            tc.tile_pool(name="const", bufs=1) as const,
            tc.tile_pool(name="srep", bufs=1) as srep_p,
            tc.tile_pool(name="schunk", bufs=NCORES) as schunk_p,
            tc.tile_pool(name="h1", bufs=1) as h1_p,
            tc.tile_pool(name="zsb", bufs=1) as z_p,
            tc.tile_pool(name="zchunk", bufs=10) as zchunk_p,
            tc.tile_pool(name="astream", bufs=16) as astream,
            tc.tile_pool(name="afull", bufs=8) as afull_p,
            tc.tile_pool(name="outsb", bufs=1) as outsb_p,
            tc.tile_pool(name="dram", bufs=1, space="DRAM") as dram,
        ):
            # ---- constants into SBUF, in consumption order: w1 and the
            # core's own x-blocks first (S-own + its gather doorbell go out
            # ~6us into the kernel), then the replicated x stream.
            w1_sb = const.tile([P, CB, NHID], fp8)
            nc.sync.dma_start(
                w1_sb[:], w1_e[:].rearrange("(cb p) j -> p cb j", p=P))
            xTo_sb = const.tile([P, CB, GP * P], fp8)
            nc.sync.dma_start(
                xTo_sb[:], xTo_e[:].rearrange("(cb p) i -> p cb i", p=P))
            NRC = NREP * NCORES * P  # replicated x columns (6144)
            xTr_sb = const.tile([P, CB, NRC], fp8)
            xTr_r = xTr_e[:].rearrange("(cb p) i -> p cb i", p=P)
            MG = 8  # replicated-x DMA granularity (~0.75MB per transfer)
            for g in range(MG):
                nc.sync.dma_start(
                    xTr_sb[:, :, g * (NRC // MG):(g + 1) * (NRC // MG)],
                    xTr_r[:, :, g * (NRC // MG):(g + 1) * (NRC // MG)])
            w2_sb = const.tile([P, JB, OUTD], bf16)
            nc.sync.dma_start(
                w2_sb[:], w2_e[:].rearrange("(jb p) o -> p jb o", p=P))
            b1p_sb = const.tile([P, JB], f32)
            nc.sync.dma_start(b1p_sb[:], b1p_e[:])
            bnsc_sb = const.tile([P, OB], f32)
            nc.sync.dma_start(bnsc_sb[:], bnsc_e[:])
            bnbi_sb = const.tile([P, OB], f32)
            nc.sync.dma_start(bnbi_sb[:], bnbi_e[:])

            # ---- DRAM bounce buffers for the collectives
            s_bounce = dram.tile([GP * P, NHID], fp8, name="sb")
            s_all = dram.tile([GP * P * NCORES, NHID], fp8,
                              addr_space="Shared", name="sa")
            RPC = ROWS // GC  # rows bounced per Z chunk (512)
            z_bounce = [dram.tile([RPC, OUTD], fp8, name=f"zb{c}")
                        for c in range(GC)]
            z_all = [dram.tile([RPC * NCORES, OUTD], fp8,
                               addr_space="Shared", name=f"za{c}")
                     for c in range(GC)]

            # ---- phase 1: S = x @ (8*W1) / 8, fp8 DoubleRow over cb-pairs
            # (256-feature contraction per matmul). Own blocks first ->
            # bounce -> gather; then the 48 replicated blocks.
            s_own = srep_p.tile([P, GP, NHID], fp8)
            s_rep = srep_p.tile([P, NREP * NCORES, NHID], fp8)

            def s_block(ps1, x_sb, ib_src, s_dst):
                ps = ps1.tile([P, NHID], f32, tag="s")
                for cp in range(CB // 2):
                    cb0 = 2 * cp
                    nc.tensor.matmul(
                        ps[:],
                        x_sb[:, cb0:cb0 + 2, ib_src * P:(ib_src + 1) * P],
                        w1_sb[:, cb0:cb0 + 2, :],
                        start=(cp == 0), stop=(cp == CB // 2 - 1),
                        perf_mode=DR,
                    )
                nc.scalar.activation(s_dst, ps[:], AF.Copy, scale=0.125)

            with tc.tile_pool(name="ps1", bufs=2, space="PSUM") as ps1:
                for ib in range(GP):
                    s_block(ps1, xTo_sb, ib, s_own[:, ib, :])
                    nc.sync.dma_start(
                        s_bounce[ib * P:(ib + 1) * P, :], s_own[:, ib, :])
                allgather(s_bounce, s_all)
                for r in range(NREP * NCORES):
                    s_block(ps1, xTr_sb, r, s_rep[:, r, :])

            # gathered-S staging: core-block k -> GP m-tiles {8k, 8k+1}
            s_gat = [None] * NCORES

            def stage_s(k):
                tile_ = schunk_p.tile([P, GP, NHID], fp8, tag="schunk")
                nc.sync.dma_start(
                    tile_[:],
                    s_all[k * GP * P:(k + 1) * GP * P, :]
                    .rearrange("(t p) j -> p t j", p=P))
                s_gat[k] = tile_

            h1T = h1_p.tile([P, JB, ROWS], bf16)
            z_sb = z_p.tile([P, IB, OUTD], fp8)

            # ---- phase 2+3, i-half pass ih: accumulate h1T half, emit z
            # half, and fire the Z allgather chunk for that half mid-kernel.
            # DoubleRow: each matmul consumes a PAIR of m-tiles (256-row
            # contraction). Replicated pairs first (they need no gather);
            # the gathered pairs {8k, 8k+1} run last.
            def l1_pass(ih, ps2, ps3):
                psum_h = [ps2.tile([P, HF], f32, name=f"ph{jb}_{ih}",
                                   tag=f"ph{jb}")
                          for jb in range(JB)]
                n_pairs = MT // 2  # 32 DoubleRow matmuls per jb
                n_emitted = 0

                def dr_pair(s_src, mt0, a_sl):
                    nonlocal n_emitted
                    for jb in range(JB):
                        nc.tensor.matmul(
                            psum_h[jb][:],
                            s_src[:, :, jb * P:(jb + 1) * P],
                            a_sl,
                            start=(n_emitted == 0),
                            stop=(n_emitted == n_pairs - 1),
                            perf_mode=DR,
                        )
                    n_emitted += 1

                # replicated part: per core-group k, m-tiles 8k+GP .. 8k+7
                # as one 4-wide a_tile per 4 m-tiles (pairs of pairs)
                for k in range(NCORES):
                    r0 = NREP * k  # s_rep index of m-tile 8k+GP
                    mt = 8 * k + GP
                    a4 = astream.tile([P, 4, HF], fp8, tag="a4")
                    nc.sync.dma_start(
                        a4[:],
                        ifadjT_e[mt * P:(mt + 4) * P,
                                 ih * HF:(ih + 1) * HF]
                        .rearrange("(four p) f -> p four f", p=P))
                    dr_pair(s_rep[:, r0:r0 + 2, :], mt, a4[:, 0:2, :])
                    dr_pair(s_rep[:, r0 + 2:r0 + 4, :], mt + 2, a4[:, 2:4, :])
                # gathered part: per core-group k, m-tiles 8k .. 8k+GP-1
                for k in range(NCORES):
                    if ih == 0:
                        stage_s(k)
                    mt = 8 * k
                    a4 = astream.tile([P, 4, HF], fp8, tag="a4")
                    nc.sync.dma_start(
                        a4[:],
                        ifadjT_e[mt * P:(mt + 4) * P,
                                 ih * HF:(ih + 1) * HF]
                        .rearrange("(four p) f -> p four f", p=P))
                    dr_pair(s_gat[k][:, 0:2, :], mt, a4[:, 0:2, :])
                    dr_pair(s_gat[k][:, 2:4, :], mt + 2, a4[:, 2:4, :])
                # epilogue: relu+bias into h1T half
                for jb in range(JB):
                    nc.scalar.activation(
                        h1T[:, jb, ih * HF:(ih + 1) * HF],
                        psum_h[jb][:], AF.Relu,
                        bias=b1p_sb[:, jb:jb + 1])
                # z for this half's i-blocks, bounce, gather chunk ih
                for t in range(IB // IH):
                    ib = ih * (IB // IH) + t
                    ps = ps3.tile([P, OUTD], f32, tag="z")
                    for jb in range(JB):
                        nc.tensor.matmul(
                            ps[:],
                            h1T[:, jb, ib * P:(ib + 1) * P],
                            w2_sb[:, jb, :],
                            start=(jb == 0), stop=(jb == JB - 1),
                        )
                    nc.scalar.activation(z_sb[:, ib, :], ps[:], AF.Copy)
                    nc.sync.dma_start(
                        z_bounce[ih][t * P:(t + 1) * P, :], z_sb[:, ib, :])
                allgather(z_bounce[ih], z_all[ih])

            with (
                tc.tile_pool(name="ps2", bufs=1, space="PSUM") as ps2,
                tc.tile_pool(name="ps3", bufs=2, space="PSUM") as ps3,
            ):
                for ih in range(IH):
                    l1_pass(ih, ps2, ps3)

            # ---- phase 4: outT[o, i] = sum_m Z[m, o] * adjT[m, i], BN fused
            # Z-chunk-major traversal: chunk c holds m-tiles {8k + 4c + q}.
            # DoubleRow pairs of m-tiles, fp8 both operands.
            outT_sb = outsb_p.tile([P, OB, ROWS], f32)
            with tc.tile_pool(name="ps4", bufs=1, space="PSUM") as ps4:
                psum_o = [[ps4.tile([P, HF], f32, name=f"po{ob}_{ih}",
                                    tag=f"po{ob}_{ih}")
                           for ih in range(IH)] for ob in range(OB)]
                first = True
                for c in range(GC):
                    for k in range(NCORES):
                        zc_sb = zchunk_p.tile([P, QT, OUTD], fp8,
                                              tag="zchunk")
                        nc.sync.dma_start(
                            zc_sb[:],
                            z_all[c][k * QT * P:(k + 1) * QT * P, :]
                            .rearrange("(t p) o -> p t o", p=P))
                        final_grp = (c == GC - 1 and k == NCORES - 1)
                        if not final_grp:
                            for qp in range(QT // 2):
                                q0 = 2 * qp
                                mt = 8 * k + 4 * c + q0
                                a_tile = afull_p.tile([P, 2, ROWS], fp8,
                                                      tag="afull")
                                nc.sync.dma_start(
                                    a_tile[:],
                                    adjT_e[mt * P:(mt + 2) * P, :]
                                    .rearrange("(two p) r -> p two r", p=P))
                                for ob in range(OB):
                                    for ih in range(IH):
                                        nc.tensor.matmul(
                                            psum_o[ob][ih][:],
                                            zc_sb[:, q0:q0 + 2,
                                                  ob * P:(ob + 1) * P],
                                            a_tile[:, :,
                                                   ih * HF:(ih + 1) * HF],
                                            start=first, stop=False,
                                            perf_mode=DR,
                                        )
                                first = False
                        else:
                            # last group: finish ob=0's accumulators first so
                            # their eviction overlaps ob=1's final matmuls
                            a_tiles = []
                            for qp in range(QT // 2):
                                q0 = 2 * qp
                                mt = 8 * k + 4 * c + q0
                                a_tile = afull_p.tile([P, 2, ROWS], fp8,
                                                      tag="afull")
                                nc.sync.dma_start(
                                    a_tile[:],
                                    adjT_e[mt * P:(mt + 2) * P, :]
                                    .rearrange("(two p) r -> p two r", p=P))
                                a_tiles.append(a_tile)
                            for ob in range(OB):
                                for qp in range(QT // 2):
                                    q0 = 2 * qp
                                    for ih in range(IH):
                                        nc.tensor.matmul(
                                            psum_o[ob][ih][:],
                                            zc_sb[:, q0:q0 + 2,
                                                  ob * P:(ob + 1) * P],
                                            a_tiles[qp][:, :,
                                                        ih * HF:(ih + 1) * HF],
                                            start=False,
                                            stop=(qp == QT // 2 - 1),
                                            perf_mode=DR,
                                        )
                # fused BN affine on PSUM evict: out = psum*scale + bias
                for ob in range(OB):
                    for ih in range(IH):
                        nc.vector.tensor_scalar(
                            outT_sb[:, ob, ih * HF:(ih + 1) * HF],
                            psum_o[ob][ih][:],
                            bnsc_sb[:, ob:ob + 1],
                            bnbi_sb[:, ob:ob + 1],
                            mybir.AluOpType.mult,
                            mybir.AluOpType.add)
                        nc.sync.dma_start(
                            out_e[ob * P:(ob + 1) * P,
                                  ih * HF:(ih + 1) * HF],
                            outT_sb[:, ob, ih * HF:(ih + 1) * HF])

    nc.compile()
    return nc


def _get_nc():
    if "nc" not in _cache:
        _cache["nc"] = _build()
    return _cache["nc"]


def kernel(x, IFadj, adj, W1, b1, W2, b2, bn_gamma, bn_beta, bn_mean, bn_var):
    from concourse.bass_utils import run_bass_kernel_spmd

    x = np.asarray(x, dtype=np.float32)
    IFadj = np.asarray(IFadj, dtype=np.float32)
    adj = np.asarray(adj, dtype=np.float32)
    W1 = np.asarray(W1, dtype=np.float32)
    b1 = np.asarray(b1, dtype=np.float32)
    W2 = np.asarray(W2, dtype=np.float32)
    b2 = np.asarray(b2, dtype=np.float32)
    bn_gamma = np.asarray(bn_gamma, dtype=np.float32)
    bn_beta = np.asarray(bn_beta, dtype=np.float32)
    bn_mean = np.asarray(bn_mean, dtype=np.float32)
    bn_var = np.asarray(bn_var, dtype=np.float32)

    # host-side prep: shard rows, transpose for PE lhsT layout, cast.
    # IFadj is mean-shifted before the fp8 cast; the exact rank-1
    # correction 0.5*colsum(x@W1) = 0.5*colsum(x)@W1 goes into the relu
    # bias. W1 is pre-scaled 8x into the e4m3 normal range (the PSUM
    # evict scales by 1/8); W2 is halved so |Z| stays well below the TRN
    # e4m3 max (240); the BN scale is doubled to compensate.
    w1b = (8.0 * W1).astype(_FP8)
    w2b = (0.5 * W2).astype(_BF16)
    b1_eff = b1 + 0.5 * (x.sum(axis=0, dtype=np.float64) @
                         W1.astype(np.float64)).astype(np.float32)
    b1p = np.ascontiguousarray(b1_eff.reshape(JB, P).T)  # [P, JB]
    inv = bn_gamma / np.sqrt(bn_var + BN_EPS)
    bias_tot = b2 * inv + bn_beta - bn_mean * inv
    bnsc = np.ascontiguousarray((2.0 * inv).reshape(OB, P).T)  # [P, OB]
    bnbi = np.ascontiguousarray(bias_tot.reshape(OB, P).T)     # [P, OB]

    # replicated x blocks: m-tiles {8k+GP .. 8k+7} for every k, r-major
    xTr = np.ascontiguousarray(np.concatenate(
        [x[k * ROWS + GP * P:(k + 1) * ROWS] for k in range(NCORES)]
    ).T).astype(_FP8)
    B = IFadj - 0.5  # zero-mean shift: 4x smaller fp8 quantization power

    in_maps = []
    for k in range(NCORES):
        r0, r1 = k * ROWS, (k + 1) * ROWS
        in_maps.append({
            "xTo": np.ascontiguousarray(
                x[r0:r0 + GP * P].T).astype(_FP8),
            "xTr": xTr,
            "ifadjT": np.ascontiguousarray(B[r0:r1].T).astype(_FP8),
            "adjT": np.ascontiguousarray(adj[r0:r1].T).astype(_FP8),
            "w1": w1b,
            "w2": w2b,
            "b1p": b1p,
            "bnsc": bnsc,
            "bnbi": bnbi,
        })

    global _last_in_maps
    _last_in_maps = in_maps

    nc = _get_nc()
    try:
        res = run_bass_kernel_spmd(nc, in_maps, list(range(NCORES)))
    except Exception:
        # transient device wedge (NRT_EXEC_UNIT_UNRECOVERABLE etc.) --
        # a straight retry has been observed to recover
        import time
        time.sleep(2.0)
        res = run_bass_kernel_spmd(nc, in_maps, list(range(NCORES)))
    # per-core output is outT [OUTD, ROWS]; transpose back and stack rows
    return np.concatenate(
        [np.ascontiguousarray(res.results[k]["out"].T)
         for k in range(NCORES)], axis=0)
